# revision 8
# baseline (speedup 1.0000x reference)
"""Trainium2 Bass kernel for nn_AttentionHead (conv3x3 -> x*am pooled -> fc6 -> fc7).

Sharding over 8 NeuronCores (single SPMD launch, 2 small collectives):
  - conv: data-parallel over batch (8 batches/core). dx-grouped matmuls:
    lhsT [128cin, 96=(3dx x 32cout)], dy absorbed via rhs window offset dy*16.
    kc-major loop interleaves 4 batch-pair PSUM chains (48 matmuls each) so
    the PE stays saturated while x tiles stream in; DVE combines the 3 dx
    strips with +dx column shifts.
  - AllGather of am.T (bf16, ~100KB/core in) so every core sees all 64 batches.
  - pooled einsum: tensor-parallel over CIN (256 ch/core); host pre-transposes
    the x chunk to [hw, cin]; 4 batches col-tiled into one PSUM bank pair.
  - fc6 with a column chunk of w6 (host pre-permuted+transposed, bf16),
    r-halves col-tiled onto PE column strips -> partial h6; AllReduce (bf16,
    128KB); +b6, relu.
  - fc7 computed fully on every core; host takes core 0's output.
All matmuls in bf16 with fp32 PSUM accumulation.
"""

import numpy as np
import ml_dtypes

import concourse.bass as bass
import concourse.bacc as bacc
import concourse.mybir as mybir
from concourse import tile
from concourse.bass_utils import run_bass_kernel_spmd
from concourse.masks import make_identity

F32 = mybir.dt.float32
BF16 = mybir.dt.bfloat16
NCORES = 8

_NC = None


def build_module(reps=1, trace_sim=False):
    nc = bacc.Bacc(None, target_bir_lowering=False)
    xbp = nc.dram_tensor("xbp", [16, 128, 8, 256], BF16, kind="ExternalInput")
    xct = nc.dram_tensor("xct", [16, 98, 4, 2, 256], BF16, kind="ExternalInput")
    cw = nc.dram_tensor("cw", [3, 128, 16, 96], BF16, kind="ExternalInput")
    cb = nc.dram_tensor("cb", [32, 1], F32, kind="ExternalInput")
    w6t = nc.dram_tensor("w6t", [16, 128, 4, 1024], BF16, kind="ExternalInput")
    b6s = nc.dram_tensor("b6s", [128, 512], F32, kind="ExternalInput")
    w7t = nc.dram_tensor("w7t", [1024, 1024], BF16, kind="ExternalInput")
    b7s = nc.dram_tensor("b7s", [128, 512], F32, kind="ExternalInput")
    outp = nc.dram_tensor("outp", [64, 1024], F32, kind="ExternalOutput")

    RG = [list(range(NCORES))]
    Relu = mybir.ActivationFunctionType.Relu
    Ident = mybir.ActivationFunctionType.Identity

    with tile.TileContext(nc, num_cores=NCORES, trace_sim=trace_sim) as tc:
        with (
            tc.tile_pool(name="consts", bufs=1) as consts,
            tc.tile_pool(name="dram", bufs=1, space="DRAM") as dram,
            tc.tile_pool(name="ps_t", bufs=2, space="PSUM") as ps_t,
            tc.tile_pool(name="ps_pool", bufs=1, space="PSUM") as ps_pool,
            tc.tile_pool(name="sb_main", bufs=1) as sb_main,
            tc.tile_pool(name="xbp_pool", bufs=4) as xbp_pool,
            tc.tile_pool(name="amio", bufs=4) as amio,
            tc.tile_pool(name="xct_pool", bufs=6) as xct_pool,
            tc.tile_pool(name="w6_pool", bufs=12) as w6_pool,
            tc.tile_pool(name="small", bufs=2) as small,
        ):
            ident = consts.tile([128, 128], BF16)
            make_identity(nc, ident[:])
            cw_sb = consts.tile([128, 3, 16, 96], BF16)
            nc.sync.dma_start(cw_sb[:], cw[:].rearrange("dy p kc m -> p dy kc m"))
            cb_sb = consts.tile([32, 1], F32)
            nc.sync.dma_start(cb_sb[:], cb[:])
            b6_sb = consts.tile([128, 512], F32)
            nc.sync.dma_start(b6_sb[:], b6s[:])
            b7_sb = consts.tile([128, 512], F32)
            nc.sync.dma_start(b7_sb[:], b7s[:])
            w7a = consts.tile([128, 4, 1024], BF16)
            nc.sync.dma_start(w7a[:], w7t[:].rearrange("(q p) r -> p q r", q=8)[:, 0:4, :])
            w7b = consts.tile([128, 4, 1024], BF16)
            nc.sync.dma_start(w7b[:], w7t[:].rearrange("(q p) r -> p q r", q=8)[:, 4:8, :])

            for rep in range(reps):
                # feat transposed for fc6: [p=i%128, ih, g, (j,o)]
                featT = sb_main.tile([128, 2, 16, 128], BF16)

                ag_in = dram.tile([8, 196, 32], BF16)
                ag_out = dram.tile([64, 196, 32], BF16, addr_space="Shared")
                ar_in = dram.tile([64, 1024], F32)
                ar_out = dram.tile([64, 1024], F32, addr_space="Shared")

                # ---------------- Phase 1: conv for own 8 batches ----------------
                # 4 batch-pair PSUM chains advance together, kc-major, so each
                # x tile is consumed by 12 matmuls right after it lands.
                with tc.tile_pool(name="ps_conv", bufs=1, space="PSUM") as ps_conv:
                    ps4 = [ps_conv.tile([128, 2, 224], F32, name=f"cps{pp}")
                           for pp in range(4)]
                    for kc in range(16):
                        t = xbp_pool.tile([128, 8, 256], BF16, tag="xbp")
                        nc.sync.dma_start(t[:], xbp[kc])
                        for dy in range(3):
                            for pp in range(4):
                                nc.tensor.matmul(
                                    ps4[pp][0:96, :, :],
                                    cw_sb[:, dy, kc, :],
                                    t[:, 2 * pp:2 * pp + 2, dy * 16:dy * 16 + 224],
                                    start=(kc == 0 and dy == 0),
                                    stop=(kc == 15 and dy == 2),
                                )
                    for pp in range(4):
                        amacc = small.tile([32, 2, 14, 14], F32, tag="amacc")
                        S4 = ps4[pp][:].rearrange("p a (r c) -> p a r c", r=14)
                        nc.vector.tensor_copy(amacc[:], S4[0:32, :, :, 0:14])
                        nc.vector.tensor_add(amacc[:], amacc[:], S4[32:64, :, :, 1:15])
                        nc.vector.tensor_add(amacc[:], amacc[:], S4[64:96, :, :, 2:16])
                        ambf = amio.tile([32, 2, 14, 14], BF16, tag="ambf")
                        nc.scalar.activation(ambf[:], amacc[:], Ident, bias=cb_sb[:])
                        for b2 in range(2):
                            amt = amio.tile([98, 2, 32], BF16, tag="amt")
                            flat = ambf[:, b2, :, :].rearrange("p r c -> p (r c)")
                            for h in range(2):
                                tp = ps_t.tile([98, 32], BF16, tag="tp")
                                nc.tensor.transpose(
                                    tp[:], flat[:, h * 98:(h + 1) * 98], ident[:32, :32]
                                )
                                nc.vector.tensor_copy(amt[:, h, :], tp[:])
                            b = 2 * pp + b2
                            nc.sync.dma_start(
                                ag_in[b, :, :].rearrange("(h p) o -> p h o", h=2), amt[:]
                            )

                # ---------------- Phase 2: AllGather am.T ----------------
                nc.gpsimd.collective_compute(
                    "AllGather", mybir.AluOpType.bypass,
                    replica_groups=RG, ins=[ag_in[:]], outs=[ag_out[:]],
                )
                # all 64 batches' am.T in SBUF: [p, h, b, o]
                amT = sb_main.tile([98, 2, 64, 32], BF16)
                for h in range(2):
                    nc.sync.dma_start(
                        amT[:, h, :, :],
                        ag_out[:, h * 98:(h + 1) * 98, :].rearrange("b p o -> p b o"),
                    )

                # ---- Phase 3: pooled einsum, 4 batches col-tiled per PSUM bank ----
                for g in range(16):
                    xv = xct_pool.tile([98, 4, 2, 256], BF16, tag="xv")
                    nc.sync.dma_start(xv[:], xct[g])
                    psA = ps_pool.tile([128, 256], F32, tag="ppsA")
                    psB = ps_pool.tile([128, 256], F32, tag="ppsB")
                    for j in range(4):
                        b = 4 * g + j
                        nc.tensor.matmul(
                            psA[32 * j:32 * j + 32, :], amT[:, 0, b, :], xv[:, j, 0, :],
                            start=True, stop=True, tile_position=(0, 32 * j),
                        )
                        nc.tensor.matmul(
                            psB[32 * j:32 * j + 32, :], amT[:, 1, b, :], xv[:, j, 1, :],
                            start=True, stop=True, tile_position=(0, 32 * j),
                        )
                    stage = small.tile([128, 256], F32, tag="po4")
                    nc.scalar.activation(stage[:], psA[:], Ident)
                    stage2 = small.tile([128, 256], BF16, tag="po4b")
                    nc.vector.tensor_add(stage2[:], stage[:], psB[:])
                    # transpose [(j,o), i-half] -> [i-half, (j,o)] and scatter
                    # into featT[:, kc=(ih*32+o), b=4g+j]
                    for ih in range(2):
                        tp = ps_t.tile([128, 128], BF16, tag="tp")
                        nc.tensor.transpose(
                            tp[:], stage2[:, ih * 128:(ih + 1) * 128], ident[:]
                        )
                        nc.vector.tensor_copy(featT[:, ih, g, :], tp[:])

                # ---- Phase 5: fc6 (col-tiled r-halves) + AllReduce + relu ----
                with tc.tile_pool(name="ps_acc", bufs=2, space="PSUM") as ps_acc:
                    h6A = ps_acc.tile([128, 512], F32, tag="acc", name="h6A")
                    h6B = ps_acc.tile([128, 512], F32, tag="acc", name="h6B")
                    for kc4 in range(16):
                        wt = w6_pool.tile([128, 4, 1024], BF16, tag="w6")
                        nc.sync.dma_start(wt[:], w6t[kc4])
                        for q in range(4):
                            kc = 4 * kc4 + q
                            ih, o = kc // 32, kc % 32
                            lhs = featT[:].rearrange(
                                "p h g (j o) -> p h g o j", j=4
                            )[:, ih, :, o, :]
                            nc.tensor.matmul(
                                h6A[0:64, :], lhs, wt[:, q, 0:512],
                                start=(kc == 0), stop=(kc == 63), tile_position=(0, 0),
                            )
                            nc.tensor.matmul(
                                h6B[64:128, :], lhs, wt[:, q, 512:1024],
                                start=(kc == 0), stop=(kc == 63), tile_position=(0, 64),
                            )
                    # h6sb rows 0-63: r 0-511 (h6A), rows 64-127: r 512-1023 (h6B)
                    h6sb = sb_main.tile([128, 512], F32)
                    nc.vector.tensor_add(h6sb[0:64, :], h6A[0:64, :], b6_sb[0:64, :])
                    nc.vector.tensor_add(h6sb[64:128, :], h6B[64:128, :], b6_sb[64:128, :])
                    nc.sync.dma_start(
                        ar_in[:].rearrange("b (s r) -> s b r", s=2), h6sb[:]
                    )
                    nc.gpsimd.collective_compute(
                        "AllReduce", mybir.AluOpType.add,
                        replica_groups=RG, ins=[ar_in[:]], outs=[ar_out[:]],
                    )
                    h6r = sb_main.tile([64, 1024], F32)
                    nc.sync.dma_start(h6r[:], ar_out[:])
                    h6a = sb_main.tile([64, 1024], BF16)
                    nc.scalar.activation(h6a[:], h6r[:], Relu)

                    # ------------- Phase 6: fc7 on all 64 batches -------------
                    h7T = sb_main.tile([128, 8, 64], BF16)
                    for k7 in range(8):
                        tp = ps_t.tile([128, 64], BF16, tag="tp")
                        nc.tensor.transpose(
                            tp[:], h6a[:, k7 * 128:(k7 + 1) * 128], ident[:64, :64]
                        )
                        nc.vector.tensor_copy(h7T[:, k7, :], tp[:])
                    opsA = ps_acc.tile([128, 512], F32, tag="acc", name="opsA")
                    opsB = ps_acc.tile([128, 512], F32, tag="acc", name="opsB")
                    for k7 in range(8):
                        wsrc = w7a if k7 < 4 else w7b
                        nc.tensor.matmul(
                            opsA[0:64, :], h7T[:, k7, :], wsrc[:, k7 % 4, 0:512],
                            start=(k7 == 0), stop=(k7 == 7), tile_position=(0, 0),
                        )
                        nc.tensor.matmul(
                            opsB[64:128, :], h7T[:, k7, :], wsrc[:, k7 % 4, 512:1024],
                            start=(k7 == 0), stop=(k7 == 7), tile_position=(0, 64),
                        )
                    t12 = small.tile([128, 512], F32, tag="t12")
                    nc.vector.tensor_add(t12[0:64, :], opsA[0:64, :], b7_sb[0:64, :])
                    nc.vector.tensor_add(t12[64:128, :], opsB[64:128, :], b7_sb[64:128, :])
                    osb = sb_main.tile([128, 512], F32)
                    nc.scalar.activation(osb[:], t12[:], Relu)
                    nc.sync.dma_start(
                        outp[:].rearrange("b (s r) -> s b r", s=2), osb[:]
                    )

    nc.compile()
    return nc


def _bf(a):
    return np.ascontiguousarray(a).astype(ml_dtypes.bfloat16)


def _f32(a):
    return np.ascontiguousarray(a).astype(np.float32)


def prep_inputs(x, conv_w, conv_b, w6, b6, w7, b7):
    x = np.asarray(x, np.float32)
    conv_w = np.asarray(conv_w, np.float32)
    conv_b = np.asarray(conv_b, np.float32)
    w6 = np.asarray(w6, np.float32)
    b6 = np.asarray(b6, np.float32)
    w7 = np.asarray(w7, np.float32)
    b7 = np.asarray(b7, np.float32)

    xp = np.zeros((64, 2048, 16, 16), np.float32)
    xp[:, :, 1:15, 1:15] = x
    xpg = xp.reshape(64, 2048, 256)
    w6r = w6.reshape(1024, 2048, 32)
    # cw[dy, c%128, kc, dx*32+o] = conv_w[o, c, dy, dx]/196
    cwt = (conv_w / 196.0).reshape(32, 16, 128, 3, 3).transpose(3, 2, 1, 4, 0)
    cw = np.ascontiguousarray(cwt).reshape(3, 128, 16, 96)
    cb = (conv_b / 196.0).reshape(32, 1)
    # stacked bias layouts: rows 0-63 get [0:512], rows 64-127 get [512:1024]
    b6s_ = np.empty((128, 512), np.float32)
    b6s_[0:64, :] = (b6[0:512] / NCORES)[None, :]
    b6s_[64:128, :] = (b6[512:1024] / NCORES)[None, :]
    b7s_ = np.empty((128, 512), np.float32)
    b7s_[0:64, :] = b7[0:512][None, :]
    b7s_[64:128, :] = b7[512:1024][None, :]
    w7t = w7.T

    cw_b = _bf(cw)
    cb_f = _f32(cb)
    b6_f = _f32(b6s_)
    b7_f = _f32(b7s_)
    w7_b = _bf(w7t)

    in_maps = []
    for c in range(NCORES):
        bs = slice(8 * c, 8 * c + 8)
        i0 = 256 * c
        # xbp[kc, c%128, b, pix]
        xbp_ = np.ascontiguousarray(
            xpg[bs].reshape(8, 16, 128, 256).transpose(1, 2, 0, 3))
        xs = x[:, i0:i0 + 256].reshape(64, 256, 196)
        xct_o = xs.transpose(0, 2, 1).reshape(64, 2, 98, 256).transpose(0, 2, 1, 3)
        xct_ = xct_o.reshape(16, 4, 98, 2, 256).transpose(0, 2, 1, 3, 4)
        w6c = w6r[:, i0:i0 + 256, :]  # [r, il, o]
        # row order: kc = (il//128)*32 + o, p = il%128
        w6t_o = np.ascontiguousarray(
            w6c.reshape(1024, 2, 128, 32).transpose(1, 3, 2, 0)
        ).reshape(8192, 1024)
        w6t_ = w6t_o.reshape(16, 4, 128, 1024).transpose(0, 2, 1, 3)
        in_maps.append(dict(
            xbp=_bf(xbp_), xct=_bf(xct_), cw=cw_b, cb=cb_f,
            w6t=_bf(w6t_), b6s=b6_f, w7t=w7_b, b7s=b7_f,
        ))
    return in_maps


def run(in_maps, **kwargs):
    global _NC
    if _NC is None:
        _NC = build_module()
    return run_bass_kernel_spmd(_NC, in_maps, list(range(NCORES)), **kwargs)


def kernel(x, conv_w, conv_b, w6, b6, w7, b7):
    in_maps = prep_inputs(x, conv_w, conv_b, w6, b6, w7, b7)
    res = run(in_maps)
    return np.asarray(res.results[0]["outp"], dtype=np.float32)
